# revision 19
# baseline (speedup 1.0000x reference)
"""Trainium2 Bass kernel for supervised contrastive loss (8-core SPMD).

Math (per reference):
    f = x / max(||x||, 1e-12)            row-normalized features  [B, D]
    s = (f f^T) / TEMP                                            [B, B]
    E = exp(s) with diag zeroed
    P_i = sum_{j != i, l_j == l_i} E_ij   (positives)
    T_i = sum_{j != i} E_ij               (positives + negatives)
    loss = mean_i [ log(T_i + EPS) - log(P_i) ]

Distribution: row-block shard with an on-device AllGather. Each core
receives ONLY its own 1024 rows (1 MB bf16, chunk-major x^T layout) plus
a small label table — the host->device tunnel is the bottleneck in this
setup, so input bytes are minimized. On device, each core normalizes its
rows, AllGathers the normalized chunks over NeuronLink, and computes its
[j, m] E^T blocks with j on the partition dim so both masked reductions
are TensorEngine partition-contractions:
    PS1[c', m] = sum_j Y'[j, c'] * E[j, m]     (Y' = one-hot(labels) | ones)
row 0 of PS1 = T_m, and P_m = PS1[l_m + 1, m] (recovered with a one-hot
mask + ones-matmul). Per-core scalar partial losses are summed on host.

SPMD uniformity: every core runs the identical program. Own chunks are
processed from SBUF at iterations 0..7 (with the compile-time diagonal
kill); all 64 gathered chunks are processed uniformly afterwards, with
the 8 own-chunk duplicates neutralized by a sentinel label value in the
shipped table (their one-hot AND ones columns compare to all-zero).
"""

import numpy as np
import ml_dtypes

TEMPERATURE = 0.07
EPS = 1e-8
B = 8192
D = 512
NCORES = 8
M = B // NCORES          # 1024 rows per core
NCH = B // 128           # 64 j-chunks of 128
BCH = M // 128           # 8 chunks in the core's own block
NCLS = 100               # label classes
YC = NCLS + 1            # one-hot columns + ones column
SENT = -1000.0           # label sentinel: matches no class, zeroes Y cols
LIO = NCH + BCH + YC     # labio columns: lab_y | lab_own | iota101

FP8 = True               # ship features as float8_e3m4 (1 B/elem)
_CACHE = {}


def _enable_jax_executable_cache():
    """Persist compiled XLA executables (with the embedded NEFF) so repeat
    run_bass_kernel_spmd calls skip the per-call BIR->NEFF recompile that
    the fresh jit closure in run_bass_via_pjrt otherwise triggers."""
    try:
        import os
        import tempfile
        import jax

        d = os.path.join(tempfile.gettempdir(), "jax_exec_cache")
        os.makedirs(d, exist_ok=True)
        jax.config.update("jax_compilation_cache_dir", d)
        jax.config.update("jax_persistent_cache_min_compile_time_secs", 0)
        jax.config.update("jax_persistent_cache_min_entry_size_bytes", 0)
    except Exception:
        pass


def _build_bass(fp8: bool = FP8, n_gather: int = NCH):
    import concourse.bass as bass
    import concourse.bacc as bacc
    import concourse.tile as tile
    from concourse import mybir
    from contextlib import ExitStack

    f32 = mybir.dt.float32
    bf16 = mybir.dt.bfloat16
    xdt = mybir.dt.float8e3 if fp8 else bf16
    AF = mybir.ActivationFunctionType
    OP = mybir.AluOpType

    # partition id is unused (per-core variation is data-driven) and each
    # extra input param costs ~8ms/call in axon PJRT round trips
    nc = bacc.Bacc(num_devices=NCORES, enable_partition_id=False)

    # ---- I/O ----------------------------------------------------------
    # One flat byte param (fewer params = fewer per-call transfers):
    #   bytes [0 : XB)        xt8[k, p, dc*128+jj] = x[own+128k+jj, dc*128+p]
    #   bytes [XB : XB+LB)    labio [128, LIO] bf16:
    #     [:, 0:64]    lab_y[p, u] = labels[u*128+p], own chunks -> SENT
    #     [:, 64:72]   lab_own[p, k] = labels[own_base + 128k + p]
    #     [:, 72:173]  iota101[p, c'] = c' - 1
    XB = BCH * 128 * D       # xt8 bytes (fp8: 1 B/elem)
    LB = 128 * LIO * 2       # labio bytes (bf16)
    if fp8:
        xlab_d = nc.declare_dram_parameter(
            "xlab", [1, XB + LB], xdt, isOutput=False
        )
    else:
        xt8_d = nc.declare_dram_parameter(
            "xt8", [BCH, 128, D], xdt, isOutput=False
        )
        labio_d = nc.declare_dram_parameter(
            "labio", [128, LIO], bf16, isOutput=False
        )
    loss_d = nc.declare_dram_parameter("loss", [1, 1], f32, isOutput=True)

    with ExitStack() as ctx:
        tc = ctx.enter_context(tile.TileContext(nc))
        const = ctx.enter_context(tc.tile_pool(name="const", bufs=1))
        gp = ctx.enter_context(tc.tile_pool(name="gp", bufs=4))
        ep = ctx.enter_context(tc.tile_pool(name="ep", bufs=3))
        psum = ctx.enter_context(tc.tile_pool(name="psum", bufs=3, space="PSUM"))
        accp = ctx.enter_context(tc.tile_pool(name="accp", bufs=1, space="PSUM"))
        dram = ctx.enter_context(tc.tile_pool(name="dram", bufs=1, space="DRAM"))

        # ---- label machinery ------------------------------------------
        if fp8:
            labraw = const.tile([128, LIO * 2], xdt)
            nc.sync.dma_start(
                out=labraw[:],
                in_=xlab_d[0:1, XB : XB + LB].rearrange(
                    "o (p f) -> p (o f)", p=128
                ),
            )
            labio = labraw[:].bitcast(bf16)
        else:
            labio_t = const.tile([128, LIO], bf16)
            nc.sync.dma_start(out=labio_t[:], in_=labio_d[:])
            labio = labio_t[:]
        laby = labio[:, 0:NCH]
        labown = labio[:, NCH : NCH + BCH]
        iota101 = labio[:, NCH + BCH : LIO]
        # is_equal needs an f32 scalar AP; cast the label columns up front
        labf = const.tile([128, NCH + BCH], f32)
        nc.vector.tensor_copy(out=labf[:], in_=labio[:, 0 : NCH + BCH])
        labyf = labf[:, 0:NCH]
        labownf = labf[:, NCH : NCH + BCH]

        # Y for gathered chunks: yg[p, u, c'] = (labels[u*128+p] == c'-1)
        # for c' >= 1; col 0 (the T-sum ones column) = (label != SENT).
        yg = const.tile([128, NCH, YC], bf16)
        nc.vector.tensor_scalar(
            out=yg[:, :, 0:1].rearrange("p u o -> p (u o)"), in0=laby,
            scalar1=SENT, scalar2=None, op0=OP.not_equal,
        )
        for u in range(NCH):
            nc.vector.tensor_scalar(
                out=yg[:, u, 1:YC], in0=iota101[:, 1:YC],
                scalar1=labyf[:, u : u + 1], scalar2=None, op0=OP.is_equal,
            )

        # Y for own chunks (diag handled by affine_select on E instead)
        yo = const.tile([128, BCH, YC], bf16)
        nc.vector.memset(yo[:, :, 0:1], 1.0)
        for k in range(BCH):
            nc.vector.tensor_scalar(
                out=yo[:, k, 1:YC], in0=iota101[:, 1:YC],
                scalar1=labownf[:, k : k + 1], scalar2=None, op0=OP.is_equal,
            )

        # YblkT[c', m] = (labels[own m] == c'-1): per-chunk PE transposes
        # of yo (identity built on device with an affine_select diagonal).
        ident = const.tile([128, 128], bf16)
        nc.vector.memset(ident[:], 1.0)
        nc.gpsimd.affine_select(
            out=ident[:], in_=ident[:], pattern=[[1, 128]],
            compare_op=OP.is_equal, fill=0.0, base=0, channel_multiplier=-1,
        )
        trans_ps = psum.tile([128, M], bf16, tag="sim", name="trans_ps")
        for k in range(BCH):
            nc.tensor.transpose(
                trans_ps[0:YC, k * 128 : (k + 1) * 128], yo[:, k, :], ident[:]
            )
        yblkt = const.tile([128, M], bf16)
        nc.vector.tensor_copy(out=yblkt[0:YC, :], in_=trans_ps[0:YC, :])
        # row 0 is the transposed ones column — must not count T into P
        nc.vector.memset(yblkt[0:1, :], 0.0)

        ones101 = const.tile([128, 1], f32)
        nc.vector.memset(ones101[:], 1.0)
        bias_eps = const.tile([128, 1], f32)
        nc.vector.memset(bias_eps[:], EPS)

        # ---- own rows: load, row norms, normalize ---------------------
        if fp8:
            x8r = const.tile([128, BCH, D], xdt)
            nc.sync.dma_start(
                out=x8r[:],
                in_=xlab_d[0:1, 0:XB].rearrange(
                    "o (t p f) -> p (o t) f", t=BCH, p=128
                ),
            )
            x8 = const.tile([128, BCH, D], bf16)
            nc.vector.tensor_copy(out=x8[:], in_=x8r[:])
        else:
            x8 = const.tile([128, BCH, D], bf16)
            nc.sync.dma_start(out=x8[:], in_=xt8_d[:].rearrange("t p f -> p t f"))

        # nsq_row[0, m] = sum_d x[m, d]^2 via DVE square + accumulated
        # ones-matmul partition reductions (4 dc groups x 2 halves).
        ones_bf = const.tile([128, 1], bf16)
        nc.vector.memset(ones_bf[:], 1.0)
        x8sq = const.tile([128, BCH, D], bf16)
        nc.vector.tensor_tensor(out=x8sq[:], in0=x8[:], in1=x8[:], op=OP.mult)
        nsqrow_ps = psum.tile([128, M], f32, tag="sim", name="nsqrow_ps")
        for dc in range(4):
            for h in range(2):
                nc.tensor.matmul(
                    nsqrow_ps[0:1, h * 512 : (h + 1) * 512],
                    lhsT=ones_bf[:, 0:1],
                    rhs=x8sq[:, h * 4 : (h + 1) * 4, dc * 128 : (dc + 1) * 128],
                    start=(dc == 0),
                    stop=(dc == 3),
                )
        # 1/||x_m|| = exp(-0.5*ln(nsq)) on partition 0 of a zeroed tile,
        # broadcast to all partitions with a ones-matmul.
        lnrow = const.tile([1, M], f32)
        nc.scalar.activation(out=lnrow[:], in_=nsqrow_ps[0:1, :], func=AF.Ln)
        rowpad = const.tile([128, M], f32)
        nc.vector.memset(rowpad[:], 0.0)
        nc.scalar.activation(
            out=rowpad[0:1, :], in_=lnrow[:], func=AF.Exp, bias=0.0, scale=-0.5
        )
        ones_f = const.tile([128, 128], f32)
        nc.vector.memset(ones_f[:], 1.0)
        invnbc_ps = psum.tile([128, M], f32, tag="sim", name="invnbc_ps")
        for h in range(2):
            nc.tensor.matmul(
                invnbc_ps[:, h * 512 : (h + 1) * 512],
                lhsT=ones_f[:],
                rhs=rowpad[:, h * 512 : (h + 1) * 512],
                start=True,
                stop=True,
            )
        invnbc = const.tile([128, M], f32)
        nc.vector.tensor_copy(out=invnbc[:], in_=invnbc_ps[:])
        # normalized own-block x^T: xnt[p, dc, m] = x[m, dc*128+p]/||x_m||
        xnt = const.tile([128, 4, M], bf16)
        for dc in range(4):
            nc.vector.tensor_tensor(
                out=xnt[:, dc, :].rearrange("p (t j) -> p t j", j=128),
                in0=x8[:, :, dc * 128 : (dc + 1) * 128],
                in1=invnbc[:].rearrange("p (t j) -> p t j", j=128),
                op=OP.mult,
            )

        # ---- AllGather normalized chunks over NeuronLink --------------
        gin = dram.tile([BCH, 128, D], bf16)
        gout = dram.tile([NCH, 128, D], bf16)
        nc.gpsimd.dma_start(
            gin[:].rearrange("k p (dc jj) -> p dc k jj", jj=128),
            xnt[:, :, :].rearrange("p dc (k jj) -> p dc k jj", jj=128),
        )
        nc.gpsimd.collective_compute(
            "AllGather",
            mybir.AluOpType.bypass,
            replica_groups=[list(range(NCORES))],
            ins=[gin.opt()],
            outs=[gout.opt()],
        )

        # ---- main loop: 8 own chunks (SBUF) + 64 gathered chunks ------
        NT = BCH + n_gather
        ps1 = accp.tile([128, M], f32)  # row 0: T; rows 1..100: class sums
        for t in range(NT):
            if t < BCH:
                lhs = None
            else:
                g = gp.tile([128, D], bf16)
                nc.sync.dma_start(out=g[:], in_=gout[t - BCH])
                lhs = g[:]
            ps = psum.tile([128, M], f32, tag="sim")
            for dc in range(4):
                lhsT = (
                    xnt[:, dc, t * 128 : (t + 1) * 128]
                    if t < BCH
                    else lhs[:, dc * 128 : (dc + 1) * 128]
                )
                for h in range(2):
                    nc.tensor.matmul(
                        ps[:, h * 512 : (h + 1) * 512],
                        lhsT=lhsT,
                        rhs=xnt[:, dc, h * 512 : (h + 1) * 512],
                        start=(dc == 0),
                        stop=(dc == 3),
                    )
            e_t = ep.tile([128, M], bf16)
            nc.scalar.activation(
                out=e_t[:], in_=ps[:], func=AF.Exp, scale=float(1.0 / TEMPERATURE)
            )
            if t < BCH:
                # zero the diagonal: kill (p, m) where m - p - 128*t == 0
                nc.gpsimd.affine_select(
                    out=e_t[:], in_=e_t[:], pattern=[[1, M]],
                    compare_op=OP.not_equal, fill=0.0,
                    base=-(t * 128), channel_multiplier=-1,
                )
            yt = yo[:, t, :] if t < BCH else yg[:, t - BCH, :]
            for h in range(2):
                nc.tensor.matmul(
                    ps1[0:YC, h * 512 : (h + 1) * 512],
                    lhsT=yt,
                    rhs=e_t[:, h * 512 : (h + 1) * 512],
                    start=(t == 0),
                    stop=(t == NT - 1),
                )

        # ---- finalize: P via one-hot mask + partition reduce ----------
        maskd = const.tile([128, M], f32)
        nc.vector.tensor_tensor(
            out=maskd[0:YC, :], in0=ps1[0:YC, :], in1=yblkt[0:YC, :], op=OP.mult
        )
        pps = psum.tile([128, M], f32, tag="sim")
        for h in range(2):
            nc.tensor.matmul(
                pps[0:1, h * 512 : (h + 1) * 512],
                lhsT=ones101[0:YC, 0:1],
                rhs=maskd[0:YC, h * 512 : (h + 1) * 512],
                start=True,
                stop=True,
            )
        ln_t = const.tile([1, M], f32)
        nc.scalar.activation(
            out=ln_t[:], in_=ps1[0:1, :], func=AF.Ln, bias=bias_eps[0:1, :]
        )
        ln_p = const.tile([1, M], f32)
        nc.scalar.activation(out=ln_p[:], in_=pps[0:1, :], func=AF.Ln)
        diff = const.tile([1, M], f32)
        nc.vector.tensor_sub(out=diff[:], in0=ln_t[:], in1=ln_p[:])
        losss = const.tile([1, 1], f32)
        nc.vector.tensor_reduce(
            out=losss[:], in_=diff[:], axis=mybir.AxisListType.X, op=OP.add
        )
        nc.sync.dma_start(out=loss_d[:], in_=losss[:])

    nc.finalize()
    return nc


def _prep_inputs(features: np.ndarray, labels: np.ndarray, fp8: bool = FP8):
    """Shard the full inputs for the 8 cores (host marshalling)."""
    bf16 = ml_dtypes.bfloat16
    xdt = ml_dtypes.float8_e3m4 if fp8 else bf16
    x_bf = np.ascontiguousarray(features, dtype=np.float32).astype(xdt)
    # chunk-major x^T: xtc[u, p, dc*128+jj] = x[u*128+jj, dc*128+p]
    xtc = np.ascontiguousarray(
        x_bf.reshape(NCH, 128, 4, 128).transpose(0, 3, 2, 1)
    ).reshape(NCH, 128, D)
    del x_bf
    lab_ch = labels.astype(np.float32).reshape(NCH, 128).T  # [p, u]
    iota101 = (np.arange(YC, dtype=np.float32) - 1.0)[None, :]
    in_maps = []
    for c in range(NCORES):
        labio = np.empty((128, LIO), dtype=np.float32)
        labio[:, 0:NCH] = lab_ch
        labio[:, BCH * c : BCH * (c + 1)] = SENT
        labio[:, NCH : NCH + BCH] = lab_ch[:, BCH * c : BCH * (c + 1)]
        labio[:, NCH + BCH : LIO] = iota101
        xt8 = xtc[BCH * c : BCH * (c + 1)]
        if fp8:
            xb = xt8.view(np.uint8).ravel()
            lb = np.ascontiguousarray(labio.astype(bf16)).view(np.uint8).ravel()
            flat = np.concatenate([xb, lb]).view(xdt)[None, :]
            in_maps.append({"xlab": flat})
        else:
            in_maps.append({"xt8": xt8, "labio": labio.astype(bf16)})
    return in_maps


def kernel(features: np.ndarray, labels: np.ndarray) -> np.ndarray:
    from concourse.bass_utils import run_bass_kernel_spmd

    _enable_jax_executable_cache()
    if "nc" not in _CACHE:
        _CACHE["nc"] = _build_bass()
    nc = _CACHE["nc"]
    in_maps = _prep_inputs(features, labels)
    res = run_bass_kernel_spmd(nc, in_maps, list(range(NCORES)))
    total = sum(float(r["loss"][0, 0]) for r in res.results)
    return np.float32(total / B)


# revision 20
# speedup vs baseline: 1.2396x; 1.2396x over previous
"""Trainium2 Bass kernel for supervised contrastive loss (8-core SPMD).

Math (per reference):
    f = x / max(||x||, 1e-12)            row-normalized features  [B, D]
    s = (f f^T) / TEMP                                            [B, B]
    E = exp(s) with diag zeroed
    P_i = sum_{j != i, l_j == l_i} E_ij   (positives)
    T_i = sum_{j != i} E_ij               (positives + negatives)
    loss = mean_i [ log(T_i + EPS) - log(P_i) ]

Distribution: row-block shard with an on-device AllGather. Each core
receives ONLY its own 1024 rows (1 MB bf16, chunk-major x^T layout) plus
a small label table — the host->device tunnel is the bottleneck in this
setup, so input bytes are minimized. On device, each core normalizes its
rows, AllGathers the normalized chunks over NeuronLink, and computes its
[j, m] E^T blocks with j on the partition dim so both masked reductions
are TensorEngine partition-contractions:
    PS1[c', m] = sum_j Y'[j, c'] * E[j, m]     (Y' = one-hot(labels) | ones)
row 0 of PS1 = T_m, and P_m = PS1[l_m + 1, m] (recovered with a one-hot
mask + ones-matmul). Per-core scalar partial losses are summed on host.

SPMD uniformity: every core runs the identical program. Own chunks are
processed from SBUF at iterations 0..7 (with the compile-time diagonal
kill); all 64 gathered chunks are processed uniformly afterwards, with
the 8 own-chunk duplicates neutralized by a sentinel label value in the
shipped table (their one-hot AND ones columns compare to all-zero).
"""

import numpy as np
import ml_dtypes

TEMPERATURE = 0.07
EPS = 1e-8
B = 8192
D = 512
NCORES = 8
M = B // NCORES          # 1024 rows per core
NCH = B // 128           # 64 j-chunks of 128
BCH = M // 128           # 8 chunks in the core's own block
NCLS = 100               # label classes
YC = NCLS + 1            # one-hot columns + ones column
SENT = -1000.0           # label sentinel: matches no class, zeroes Y cols
LIO = NCH + BCH + YC     # labio columns: lab_y | lab_own | iota101

FP8 = True               # ship features as float8_e3m4 (1 B/elem)
_CACHE = {}


def _enable_jax_executable_cache():
    """Persist compiled XLA executables (with the embedded NEFF) so repeat
    run_bass_kernel_spmd calls skip the per-call BIR->NEFF recompile that
    the fresh jit closure in run_bass_via_pjrt otherwise triggers."""
    try:
        import os
        import tempfile
        import jax

        d = os.path.join(tempfile.gettempdir(), "jax_exec_cache")
        os.makedirs(d, exist_ok=True)
        jax.config.update("jax_compilation_cache_dir", d)
        jax.config.update("jax_persistent_cache_min_compile_time_secs", 0)
        jax.config.update("jax_persistent_cache_min_entry_size_bytes", 0)
    except Exception:
        pass


def _build_bass(fp8: bool = FP8, n_gather: int = NCH):
    import concourse.bass as bass
    import concourse.bacc as bacc
    import concourse.tile as tile
    from concourse import mybir
    from contextlib import ExitStack

    f32 = mybir.dt.float32
    bf16 = mybir.dt.bfloat16
    xdt = mybir.dt.float8e3 if fp8 else bf16
    AF = mybir.ActivationFunctionType
    OP = mybir.AluOpType

    # partition id is unused (per-core variation is data-driven) and each
    # extra input param costs ~8ms/call in axon PJRT round trips
    nc = bacc.Bacc(num_devices=NCORES, enable_partition_id=False)

    # ---- I/O ----------------------------------------------------------
    # One flat byte param (fewer params = fewer per-call transfers):
    #   bytes [0 : XB)        xt8[k, p, dc*128+jj] = x[own+128k+jj, dc*128+p]
    #   bytes [XB : XB+LB)    labio [128, LIO] bf16:
    #     [:, 0:64]    lab_y[p, u] = labels[u*128+p], own chunks -> SENT
    #     [:, 64:72]   lab_own[p, k] = labels[own_base + 128k + p]
    #     [:, 72:173]  iota101[p, c'] = c' - 1
    XB = BCH * 128 * D       # xt8 bytes (fp8: 1 B/elem)
    LB = 128 * LIO * 2       # labio bytes (bf16)
    if fp8:
        xlab_d = nc.declare_dram_parameter(
            "xlab", [1, XB + LB], xdt, isOutput=False
        )
    else:
        xt8_d = nc.declare_dram_parameter(
            "xt8", [BCH, 128, D], xdt, isOutput=False
        )
        labio_d = nc.declare_dram_parameter(
            "labio", [128, LIO], bf16, isOutput=False
        )
    loss_d = nc.declare_dram_parameter("loss", [1, 1], f32, isOutput=True)

    with ExitStack() as ctx:
        tc = ctx.enter_context(tile.TileContext(nc))
        const = ctx.enter_context(tc.tile_pool(name="const", bufs=1))
        gp = ctx.enter_context(tc.tile_pool(name="gp", bufs=4))
        ep = ctx.enter_context(tc.tile_pool(name="ep", bufs=3))
        psum = ctx.enter_context(tc.tile_pool(name="psum", bufs=3, space="PSUM"))
        accp = ctx.enter_context(tc.tile_pool(name="accp", bufs=1, space="PSUM"))
        dram = ctx.enter_context(tc.tile_pool(name="dram", bufs=1, space="DRAM"))

        # ---- label machinery ------------------------------------------
        if fp8:
            labraw = const.tile([128, LIO * 2], xdt)
            nc.sync.dma_start(
                out=labraw[:],
                in_=xlab_d[0:1, XB : XB + LB].rearrange(
                    "o (p f) -> p (o f)", p=128
                ),
            )
            labio = labraw[:].bitcast(bf16)
        else:
            labio_t = const.tile([128, LIO], bf16)
            nc.sync.dma_start(out=labio_t[:], in_=labio_d[:])
            labio = labio_t[:]
        laby = labio[:, 0:NCH]
        labown = labio[:, NCH : NCH + BCH]
        iota101 = labio[:, NCH + BCH : LIO]
        # is_equal needs an f32 scalar AP; cast the label columns up front
        labf = const.tile([128, NCH + BCH], f32)
        nc.vector.tensor_copy(out=labf[:], in_=labio[:, 0 : NCH + BCH])
        labyf = labf[:, 0:NCH]
        labownf = labf[:, NCH : NCH + BCH]

        # Y for gathered chunks: yg[p, u, c'] = (labels[u*128+p] == c'-1)
        # for c' >= 1; col 0 (the T-sum ones column) = (label != SENT).
        yg = const.tile([128, NCH, YC], bf16)
        nc.vector.tensor_scalar(
            out=yg[:, :, 0:1].rearrange("p u o -> p (u o)"), in0=laby,
            scalar1=SENT, scalar2=None, op0=OP.not_equal,
        )
        for u in range(NCH):
            nc.vector.tensor_scalar(
                out=yg[:, u, 1:YC], in0=iota101[:, 1:YC],
                scalar1=labyf[:, u : u + 1], scalar2=None, op0=OP.is_equal,
            )

        # Y for own chunks (diag handled by affine_select on E instead)
        yo = const.tile([128, BCH, YC], bf16)
        nc.vector.memset(yo[:, :, 0:1], 1.0)
        for k in range(BCH):
            nc.vector.tensor_scalar(
                out=yo[:, k, 1:YC], in0=iota101[:, 1:YC],
                scalar1=labownf[:, k : k + 1], scalar2=None, op0=OP.is_equal,
            )

        # YblkT[c', m] = (labels[own m] == c'-1): per-chunk PE transposes
        # of yo (identity built on device with an affine_select diagonal).
        ident = const.tile([128, 128], bf16)
        nc.vector.memset(ident[:], 1.0)
        nc.gpsimd.affine_select(
            out=ident[:], in_=ident[:], pattern=[[1, 128]],
            compare_op=OP.is_equal, fill=0.0, base=0, channel_multiplier=-1,
        )
        trans_ps = psum.tile([128, M], bf16, tag="sim", name="trans_ps")
        for k in range(BCH):
            nc.tensor.transpose(
                trans_ps[0:YC, k * 128 : (k + 1) * 128], yo[:, k, :], ident[:]
            )
        yblkt = const.tile([128, M], bf16)
        nc.vector.tensor_copy(out=yblkt[0:YC, :], in_=trans_ps[0:YC, :])
        # row 0 is the transposed ones column — must not count T into P
        nc.vector.memset(yblkt[0:1, :], 0.0)

        ones101 = const.tile([128, 1], f32)
        nc.vector.memset(ones101[:], 1.0)
        bias_eps = const.tile([128, 1], f32)
        nc.vector.memset(bias_eps[:], EPS)

        # ---- own rows: load, row norms, normalize ---------------------
        if fp8:
            x8r = const.tile([128, BCH, D], xdt)
            nc.sync.dma_start(
                out=x8r[:],
                in_=xlab_d[0:1, 0:XB].rearrange(
                    "o (t p f) -> p (o t) f", t=BCH, p=128
                ),
            )
            x8 = const.tile([128, BCH, D], bf16)
            nc.vector.tensor_copy(out=x8[:], in_=x8r[:])
        else:
            x8 = const.tile([128, BCH, D], bf16)
            nc.sync.dma_start(out=x8[:], in_=xt8_d[:].rearrange("t p f -> p t f"))

        # nsq_row[0, m] = sum_d x[m, d]^2 via DVE square + accumulated
        # ones-matmul partition reductions (4 dc groups x 2 halves).
        ones_bf = const.tile([128, 1], bf16)
        nc.vector.memset(ones_bf[:], 1.0)
        x8sq = const.tile([128, BCH, D], bf16)
        nc.vector.tensor_tensor(out=x8sq[:], in0=x8[:], in1=x8[:], op=OP.mult)
        nsqrow_ps = psum.tile([128, M], f32, tag="sim", name="nsqrow_ps")
        for dc in range(4):
            for h in range(2):
                nc.tensor.matmul(
                    nsqrow_ps[0:1, h * 512 : (h + 1) * 512],
                    lhsT=ones_bf[:, 0:1],
                    rhs=x8sq[:, h * 4 : (h + 1) * 4, dc * 128 : (dc + 1) * 128],
                    start=(dc == 0),
                    stop=(dc == 3),
                )
        # 1/||x_m|| = exp(-0.5*ln(nsq)) on partition 0 of a zeroed tile,
        # broadcast to all partitions with a ones-matmul.
        lnrow = const.tile([1, M], f32)
        nc.scalar.activation(out=lnrow[:], in_=nsqrow_ps[0:1, :], func=AF.Ln)
        rowpad = const.tile([128, M], f32)
        nc.vector.memset(rowpad[:], 0.0)
        nc.scalar.activation(
            out=rowpad[0:1, :], in_=lnrow[:], func=AF.Exp, bias=0.0, scale=-0.5
        )
        ones_f = const.tile([128, 128], f32)
        nc.vector.memset(ones_f[:], 1.0)
        invnbc_ps = psum.tile([128, M], f32, tag="sim", name="invnbc_ps")
        for h in range(2):
            nc.tensor.matmul(
                invnbc_ps[:, h * 512 : (h + 1) * 512],
                lhsT=ones_f[:],
                rhs=rowpad[:, h * 512 : (h + 1) * 512],
                start=True,
                stop=True,
            )
        invnbc = const.tile([128, M], f32)
        nc.vector.tensor_copy(out=invnbc[:], in_=invnbc_ps[:])
        # normalized own-block x^T: xnt[p, dc, m] = x[m, dc*128+p]/||x_m||
        xnt = const.tile([128, 4, M], bf16)
        for dc in range(4):
            nc.vector.tensor_tensor(
                out=xnt[:, dc, :].rearrange("p (t j) -> p t j", j=128),
                in0=x8[:, :, dc * 128 : (dc + 1) * 128],
                in1=invnbc[:].rearrange("p (t j) -> p t j", j=128),
                op=OP.mult,
            )

        # ---- AllGather normalized chunks over NeuronLink --------------
        gin = dram.tile([BCH, 128, D], bf16)
        gout = dram.tile([NCH, 128, D], bf16)
        nc.gpsimd.dma_start(
            gin[:].rearrange("k p (dc jj) -> p dc k jj", jj=128),
            xnt[:, :, :].rearrange("p dc (k jj) -> p dc k jj", jj=128),
        )
        nc.gpsimd.collective_compute(
            "AllGather",
            mybir.AluOpType.bypass,
            replica_groups=[list(range(NCORES))],
            ins=[gin.opt()],
            outs=[gout.opt()],
        )

        # ---- main loop: 8 own chunks (SBUF) + 64 gathered chunks ------
        NT = BCH + n_gather
        ps1 = accp.tile([128, M], f32)  # row 0: T; rows 1..100: class sums
        for t in range(NT):
            if t < BCH:
                lhs = None
            else:
                g = gp.tile([128, D], bf16)
                nc.sync.dma_start(out=g[:], in_=gout[t - BCH])
                lhs = g[:]
            ps = psum.tile([128, M], f32, tag="sim")
            for dc in range(4):
                lhsT = (
                    xnt[:, dc, t * 128 : (t + 1) * 128]
                    if t < BCH
                    else lhs[:, dc * 128 : (dc + 1) * 128]
                )
                for h in range(2):
                    nc.tensor.matmul(
                        ps[:, h * 512 : (h + 1) * 512],
                        lhsT=lhsT,
                        rhs=xnt[:, dc, h * 512 : (h + 1) * 512],
                        start=(dc == 0),
                        stop=(dc == 3),
                    )
            e_t = ep.tile([128, M], bf16)
            nc.scalar.activation(
                out=e_t[:], in_=ps[:], func=AF.Exp, scale=float(1.0 / TEMPERATURE)
            )
            if t < BCH:
                # zero the diagonal: kill (p, m) where m - p - 128*t == 0
                nc.gpsimd.affine_select(
                    out=e_t[:], in_=e_t[:], pattern=[[1, M]],
                    compare_op=OP.not_equal, fill=0.0,
                    base=-(t * 128), channel_multiplier=-1,
                )
            yt = yo[:, t, :] if t < BCH else yg[:, t - BCH, :]
            for h in range(2):
                nc.tensor.matmul(
                    ps1[0:YC, h * 512 : (h + 1) * 512],
                    lhsT=yt,
                    rhs=e_t[:, h * 512 : (h + 1) * 512],
                    start=(t == 0),
                    stop=(t == NT - 1),
                )

        # ---- finalize: P via one-hot mask + partition reduce ----------
        maskd = const.tile([128, M], f32)
        nc.vector.tensor_tensor(
            out=maskd[0:YC, :], in0=ps1[0:YC, :], in1=yblkt[0:YC, :], op=OP.mult
        )
        pps = psum.tile([128, M], f32, tag="sim")
        for h in range(2):
            nc.tensor.matmul(
                pps[0:1, h * 512 : (h + 1) * 512],
                lhsT=ones101[0:YC, 0:1],
                rhs=maskd[0:YC, h * 512 : (h + 1) * 512],
                start=True,
                stop=True,
            )
        ln_t = const.tile([1, M], f32)
        nc.scalar.activation(
            out=ln_t[:], in_=ps1[0:1, :], func=AF.Ln, bias=bias_eps[0:1, :]
        )
        ln_p = const.tile([1, M], f32)
        nc.scalar.activation(out=ln_p[:], in_=pps[0:1, :], func=AF.Ln)
        diff = const.tile([1, M], f32)
        nc.vector.tensor_sub(out=diff[:], in0=ln_t[:], in1=ln_p[:])
        losss = const.tile([1, 1], f32)
        nc.vector.tensor_reduce(
            out=losss[:], in_=diff[:], axis=mybir.AxisListType.X, op=OP.add
        )
        nc.sync.dma_start(out=loss_d[:], in_=losss[:])

    nc.finalize()
    # The module is frozen after finalize(), but run_bass_via_pjrt re-jits
    # per call and the bass_exec lowering re-serializes the BIR each time
    # (~9 ms). Memoize the serialization on this instance.
    raw_bir = nc.to_json_bytes()
    nc.to_json_bytes = lambda: raw_bir
    return nc


def _prep_inputs(features: np.ndarray, labels: np.ndarray, fp8: bool = FP8):
    """Shard the full inputs for the 8 cores (host marshalling)."""
    bf16 = ml_dtypes.bfloat16
    xdt = ml_dtypes.float8_e3m4 if fp8 else bf16
    x_bf = np.ascontiguousarray(features, dtype=np.float32).astype(xdt)
    # chunk-major x^T: xtc[u, p, dc*128+jj] = x[u*128+jj, dc*128+p]
    xtc = np.ascontiguousarray(
        x_bf.reshape(NCH, 128, 4, 128).transpose(0, 3, 2, 1)
    ).reshape(NCH, 128, D)
    del x_bf
    lab_ch = labels.astype(np.float32).reshape(NCH, 128).T  # [p, u]
    iota101 = (np.arange(YC, dtype=np.float32) - 1.0)[None, :]
    in_maps = []
    for c in range(NCORES):
        labio = np.empty((128, LIO), dtype=np.float32)
        labio[:, 0:NCH] = lab_ch
        labio[:, BCH * c : BCH * (c + 1)] = SENT
        labio[:, NCH : NCH + BCH] = lab_ch[:, BCH * c : BCH * (c + 1)]
        labio[:, NCH + BCH : LIO] = iota101
        xt8 = xtc[BCH * c : BCH * (c + 1)]
        if fp8:
            xb = xt8.view(np.uint8).ravel()
            lb = np.ascontiguousarray(labio.astype(bf16)).view(np.uint8).ravel()
            flat = np.concatenate([xb, lb]).view(xdt)[None, :]
            in_maps.append({"xlab": flat})
        else:
            in_maps.append({"xt8": xt8, "labio": labio.astype(bf16)})
    return in_maps


def kernel(features: np.ndarray, labels: np.ndarray) -> np.ndarray:
    from concourse.bass_utils import run_bass_kernel_spmd

    _enable_jax_executable_cache()
    if "nc" not in _CACHE:
        _CACHE["nc"] = _build_bass()
    nc = _CACHE["nc"]
    in_maps = _prep_inputs(features, labels)
    res = run_bass_kernel_spmd(nc, in_maps, list(range(NCORES)))
    total = sum(float(r["loss"][0, 0]) for r in res.results)
    return np.float32(total / B)


# revision 27
# speedup vs baseline: 1.3265x; 1.0701x over previous
"""Trainium2 Bass kernel for supervised contrastive loss (8-core SPMD).

Math (per reference):
    f = x / max(||x||, 1e-12)            row-normalized features  [B, D]
    s = (f f^T) / TEMP                                            [B, B]
    E = exp(s) with diag zeroed
    P_i = sum_{j != i, l_j == l_i} E_ij   (positives)
    T_i = sum_{j != i} E_ij               (positives + negatives)
    loss = mean_i [ log(T_i + EPS) - log(P_i) ]

Distribution: row-block shard with an on-device AllGather. Each core
receives ONLY its own 1024 rows (0.5 MB fp8 e3m4, chunk-major x^T
layout) plus a 9 KB uint8 label table — the host->device tunnel is the
bottleneck in this setup, so input bytes are minimized. On device, each core normalizes its
rows, AllGathers the normalized chunks over NeuronLink, and computes its
[j, m] E^T blocks with j on the partition dim so both masked reductions
are TensorEngine partition-contractions:
    PS1[c', m] = sum_j Y'[j, c'] * E[j, m]     (Y' = one-hot(labels) | ones)
row 0 of PS1 = T_m, and P_m = PS1[l_m + 1, m] (recovered with a one-hot
mask + ones-matmul). Per-core scalar partial losses are summed on host.

SPMD uniformity: every core runs the identical program. Own chunks are
processed from SBUF at iterations 0..7 (with the compile-time diagonal
kill); all 64 gathered chunks are processed uniformly afterwards, with
the 8 own-chunk duplicates neutralized by a sentinel label value in the
shipped table (their one-hot AND ones columns compare to all-zero).
"""

import numpy as np
import ml_dtypes

TEMPERATURE = 0.07
EPS = 1e-8
B = 8192
D = 512
NCORES = 8
M = B // NCORES          # 1024 rows per core
NCH = B // 128           # 64 j-chunks of 128
BCH = M // 128           # 8 chunks in the core's own block
NCLS = 100               # label classes
YC = NCLS + 1            # one-hot columns + ones column
SENT = 255.0             # uint8 label sentinel: matches no class, zeroes Y cols
LBC = NCH + BCH          # labio columns (uint8): lab_y | lab_own

FP8 = True               # ship features as float8_e3m4 (1 B/elem)
_CACHE = {}


def _enable_jax_executable_cache():
    """Persist compiled XLA executables (with the embedded NEFF) so repeat
    run_bass_kernel_spmd calls skip the per-call BIR->NEFF recompile that
    the fresh jit closure in run_bass_via_pjrt otherwise triggers."""
    try:
        import os
        import tempfile
        import jax

        d = os.path.join(tempfile.gettempdir(), "jax_exec_cache")
        os.makedirs(d, exist_ok=True)
        jax.config.update("jax_compilation_cache_dir", d)
        jax.config.update("jax_persistent_cache_min_compile_time_secs", 0)
        jax.config.update("jax_persistent_cache_min_entry_size_bytes", 0)
    except Exception:
        pass


def _build_bass(fp8: bool = FP8, n_gather: int = NCH):
    import concourse.bass as bass
    import concourse.bacc as bacc
    import concourse.tile as tile
    from concourse import mybir
    from contextlib import ExitStack

    f32 = mybir.dt.float32
    bf16 = mybir.dt.bfloat16
    xdt = mybir.dt.float8e3 if fp8 else bf16
    AF = mybir.ActivationFunctionType
    OP = mybir.AluOpType

    # partition id is unused (per-core variation is data-driven) and each
    # extra input param costs ~8ms/call in axon PJRT round trips
    nc = bacc.Bacc(num_devices=NCORES, enable_partition_id=False)

    # ---- I/O ----------------------------------------------------------
    # One flat byte param (fewer params = fewer per-call transfers):
    #   bytes [0 : XB)        xt8[k, p, dc*128+jj] = x[own+128k+jj, dc*128+p]
    #   bytes [XB : XB+LB)    labio [128, LBC] uint8:
    #     [:, 0:64]    lab_y[p, u] = labels[u*128+p], own chunks -> SENT
    #     [:, 64:72]   lab_own[p, k] = labels[own_base + 128k + p]
    XB = BCH * 128 * D       # xt8 bytes (fp8: 1 B/elem)
    LB = 128 * LBC           # labio bytes (uint8)
    if fp8:
        xlab_d = nc.declare_dram_parameter(
            "xlab", [1, XB + LB], xdt, isOutput=False
        )
    else:
        xt8_d = nc.declare_dram_parameter(
            "xt8", [BCH, 128, D], xdt, isOutput=False
        )
        labio_d = nc.declare_dram_parameter(
            "labio", [128, LBC], mybir.dt.uint8, isOutput=False
        )
    loss_d = nc.declare_dram_parameter("loss", [1, 1], f32, isOutput=True)

    with ExitStack() as ctx:
        tc = ctx.enter_context(tile.TileContext(nc))
        const = ctx.enter_context(tc.tile_pool(name="const", bufs=1))
        gp = ctx.enter_context(tc.tile_pool(name="gp", bufs=4))
        ep = ctx.enter_context(tc.tile_pool(name="ep", bufs=3))
        psum = ctx.enter_context(tc.tile_pool(name="psum", bufs=3, space="PSUM"))
        accp = ctx.enter_context(tc.tile_pool(name="accp", bufs=1, space="PSUM"))
        dram = ctx.enter_context(tc.tile_pool(name="dram", bufs=1, space="DRAM"))

        # ---- label machinery ------------------------------------------
        # labels ship as uint8 (sentinel 255); iota is built on device
        if fp8:
            labraw = const.tile([128, LBC], xdt)
            nc.sync.dma_start(
                out=labraw[:],
                in_=xlab_d[0:1, XB : XB + LB].rearrange(
                    "o (p f) -> p (o f)", p=128
                ),
            )
            lab_u8 = labraw[:].bitcast(mybir.dt.uint8)
        else:
            labraw8 = const.tile([128, LBC], mybir.dt.uint8)
            nc.sync.dma_start(out=labraw8[:], in_=labio_d[:])
            lab_u8 = labraw8[:]
        # is_equal needs f32 operands; cast labels up front
        labf = const.tile([128, LBC], f32)
        nc.vector.tensor_copy(out=labf[:], in_=lab_u8)
        labyf = labf[:, 0:NCH]
        labownf = labf[:, NCH : NCH + BCH]
        iota_i = const.tile([128, YC], mybir.dt.int32)
        nc.gpsimd.iota(
            iota_i[:], pattern=[[1, YC]], base=-1, channel_multiplier=0
        )
        iotaf = const.tile([128, YC], f32)
        nc.vector.tensor_copy(out=iotaf[:], in_=iota_i[:])

        # Y for gathered chunks: yg[p, u, c'] = (labels[u*128+p] == c'-1)
        # for c' >= 1; col 0 (the T-sum ones column) = (label != SENT).
        yg = const.tile([128, NCH, YC], bf16)
        nc.vector.tensor_scalar(
            out=yg[:, :, 0:1].rearrange("p u o -> p (u o)"), in0=labyf,
            scalar1=SENT, scalar2=None, op0=OP.not_equal,
        )
        for u in range(NCH):
            nc.vector.tensor_scalar(
                out=yg[:, u, 1:YC], in0=iotaf[:, 1:YC],
                scalar1=labyf[:, u : u + 1], scalar2=None, op0=OP.is_equal,
            )

        # Y for own chunks (diag handled by affine_select on E instead)
        yo = const.tile([128, BCH, YC], bf16)
        nc.vector.memset(yo[:, :, 0:1], 1.0)
        for k in range(BCH):
            nc.vector.tensor_scalar(
                out=yo[:, k, 1:YC], in0=iotaf[:, 1:YC],
                scalar1=labownf[:, k : k + 1], scalar2=None, op0=OP.is_equal,
            )

        # YblkT[c', m] = (labels[own m] == c'-1): per-chunk PE transposes
        # of yo (identity built on device with an affine_select diagonal).
        ident = const.tile([128, 128], bf16)
        nc.vector.memset(ident[:], 1.0)
        nc.gpsimd.affine_select(
            out=ident[:], in_=ident[:], pattern=[[1, 128]],
            compare_op=OP.is_equal, fill=0.0, base=0, channel_multiplier=-1,
        )
        trans_ps = psum.tile([128, M], bf16, tag="sim", name="trans_ps")
        for k in range(BCH):
            nc.tensor.transpose(
                trans_ps[0:YC, k * 128 : (k + 1) * 128], yo[:, k, :], ident[:]
            )
        yblkt = const.tile([128, M], bf16)
        nc.vector.tensor_copy(out=yblkt[0:YC, :], in_=trans_ps[0:YC, :])
        # row 0 is the transposed ones column — must not count T into P
        nc.vector.memset(yblkt[0:1, :], 0.0)

        ones101 = const.tile([128, 1], f32)
        nc.vector.memset(ones101[:], 1.0)
        bias_eps = const.tile([128, 1], f32)
        nc.vector.memset(bias_eps[:], EPS)

        # ---- own rows: load, row norms, normalize ---------------------
        if fp8:
            x8r = const.tile([128, BCH, D], xdt)
            nc.sync.dma_start(
                out=x8r[:],
                in_=xlab_d[0:1, 0:XB].rearrange(
                    "o (t p f) -> p (o t) f", t=BCH, p=128
                ),
            )
            x8 = const.tile([128, BCH, D], bf16)
            nc.vector.tensor_copy(out=x8[:], in_=x8r[:])
        else:
            x8 = const.tile([128, BCH, D], bf16)
            nc.sync.dma_start(out=x8[:], in_=xt8_d[:].rearrange("t p f -> p t f"))

        # nsq_row[0, m] = sum_d x[m, d]^2 via DVE square + accumulated
        # ones-matmul partition reductions (4 dc groups x 2 halves).
        ones_bf = const.tile([128, 1], bf16)
        nc.vector.memset(ones_bf[:], 1.0)
        x8sq = const.tile([128, BCH, D], bf16)
        nc.vector.tensor_tensor(out=x8sq[:], in0=x8[:], in1=x8[:], op=OP.mult)
        nsqrow_ps = psum.tile([128, M], f32, tag="sim", name="nsqrow_ps")
        for dc in range(4):
            for h in range(2):
                nc.tensor.matmul(
                    nsqrow_ps[0:1, h * 512 : (h + 1) * 512],
                    lhsT=ones_bf[:, 0:1],
                    rhs=x8sq[:, h * 4 : (h + 1) * 4, dc * 128 : (dc + 1) * 128],
                    start=(dc == 0),
                    stop=(dc == 3),
                )
        # 1/||x_m|| = exp(-0.5*ln(nsq)) on partition 0 of a zeroed tile,
        # broadcast to all partitions with a ones-matmul.
        lnrow = const.tile([1, M], f32)
        nc.scalar.activation(out=lnrow[:], in_=nsqrow_ps[0:1, :], func=AF.Ln)
        rowpad = const.tile([128, M], f32)
        nc.vector.memset(rowpad[:], 0.0)
        nc.scalar.activation(
            out=rowpad[0:1, :], in_=lnrow[:], func=AF.Exp, bias=0.0, scale=-0.5
        )
        ones_f = const.tile([128, 128], f32)
        nc.vector.memset(ones_f[:], 1.0)
        invnbc_ps = psum.tile([128, M], f32, tag="sim", name="invnbc_ps")
        for h in range(2):
            nc.tensor.matmul(
                invnbc_ps[:, h * 512 : (h + 1) * 512],
                lhsT=ones_f[:],
                rhs=rowpad[:, h * 512 : (h + 1) * 512],
                start=True,
                stop=True,
            )
        invnbc = const.tile([128, M], f32)
        nc.vector.tensor_copy(out=invnbc[:], in_=invnbc_ps[:])
        # normalized own-block x^T: xnt[p, dc, m] = x[m, dc*128+p]/||x_m||
        xnt = const.tile([128, 4, M], bf16)
        for dc in range(4):
            nc.vector.tensor_tensor(
                out=xnt[:, dc, :].rearrange("p (t j) -> p t j", j=128),
                in0=x8[:, :, dc * 128 : (dc + 1) * 128],
                in1=invnbc[:].rearrange("p (t j) -> p t j", j=128),
                op=OP.mult,
            )

        # ---- AllGather normalized chunks over NeuronLink --------------
        gin = dram.tile([BCH, 128, D], bf16)
        gout = dram.tile([NCH, 128, D], bf16)
        nc.gpsimd.dma_start(
            gin[:].rearrange("k p (dc jj) -> p dc k jj", jj=128),
            xnt[:, :, :].rearrange("p dc (k jj) -> p dc k jj", jj=128),
        )
        nc.gpsimd.collective_compute(
            "AllGather",
            mybir.AluOpType.bypass,
            replica_groups=[list(range(NCORES))],
            ins=[gin.opt()],
            outs=[gout.opt()],
        )

        # ---- main loop: 8 own chunks (SBUF) + 64 gathered chunks ------
        NT = BCH + n_gather
        ps1 = accp.tile([128, M], f32)  # row 0: T; rows 1..100: class sums
        for t in range(NT):
            if t < BCH:
                lhs = None
            else:
                g = gp.tile([128, D], bf16)
                nc.sync.dma_start(out=g[:], in_=gout[t - BCH])
                lhs = g[:]
            ps = psum.tile([128, M], f32, tag="sim")
            for dc in range(4):
                lhsT = (
                    xnt[:, dc, t * 128 : (t + 1) * 128]
                    if t < BCH
                    else lhs[:, dc * 128 : (dc + 1) * 128]
                )
                for h in range(2):
                    nc.tensor.matmul(
                        ps[:, h * 512 : (h + 1) * 512],
                        lhsT=lhsT,
                        rhs=xnt[:, dc, h * 512 : (h + 1) * 512],
                        start=(dc == 0),
                        stop=(dc == 3),
                    )
            e_t = ep.tile([128, M], bf16)
            nc.scalar.activation(
                out=e_t[:], in_=ps[:], func=AF.Exp, scale=float(1.0 / TEMPERATURE)
            )
            if t < BCH:
                # zero the diagonal: kill (p, m) where m - p - 128*t == 0
                nc.gpsimd.affine_select(
                    out=e_t[:], in_=e_t[:], pattern=[[1, M]],
                    compare_op=OP.not_equal, fill=0.0,
                    base=-(t * 128), channel_multiplier=-1,
                )
            yt = yo[:, t, :] if t < BCH else yg[:, t - BCH, :]
            for h in range(2):
                nc.tensor.matmul(
                    ps1[0:YC, h * 512 : (h + 1) * 512],
                    lhsT=yt,
                    rhs=e_t[:, h * 512 : (h + 1) * 512],
                    start=(t == 0),
                    stop=(t == NT - 1),
                )

        # ---- finalize: P via one-hot mask + partition reduce ----------
        maskd = const.tile([128, M], f32)
        nc.vector.tensor_tensor(
            out=maskd[0:YC, :], in0=ps1[0:YC, :], in1=yblkt[0:YC, :], op=OP.mult
        )
        pps = psum.tile([128, M], f32, tag="sim")
        for h in range(2):
            nc.tensor.matmul(
                pps[0:1, h * 512 : (h + 1) * 512],
                lhsT=ones101[0:YC, 0:1],
                rhs=maskd[0:YC, h * 512 : (h + 1) * 512],
                start=True,
                stop=True,
            )
        ln_t = const.tile([1, M], f32)
        nc.scalar.activation(
            out=ln_t[:], in_=ps1[0:1, :], func=AF.Ln, bias=bias_eps[0:1, :]
        )
        ln_p = const.tile([1, M], f32)
        nc.scalar.activation(out=ln_p[:], in_=pps[0:1, :], func=AF.Ln)
        diff = const.tile([1, M], f32)
        nc.vector.tensor_sub(out=diff[:], in0=ln_t[:], in1=ln_p[:])
        losss = const.tile([1, 1], f32)
        nc.vector.tensor_reduce(
            out=losss[:], in_=diff[:], axis=mybir.AxisListType.X, op=OP.add
        )
        nc.sync.dma_start(out=loss_d[:], in_=losss[:])

    nc.finalize()
    # The module is frozen after finalize(), but run_bass_via_pjrt re-jits
    # per call and the bass_exec lowering re-serializes the BIR each time
    # (~9 ms). Memoize the serialization on this instance.
    raw_bir = nc.to_json_bytes()
    nc.to_json_bytes = lambda: raw_bir
    return nc


def _prep_inputs(features: np.ndarray, labels: np.ndarray, fp8: bool = FP8):
    """Shard the full inputs for the 8 cores (host marshalling)."""
    bf16 = ml_dtypes.bfloat16
    xdt = ml_dtypes.float8_e3m4 if fp8 else bf16
    x_bf = np.ascontiguousarray(features, dtype=np.float32).astype(xdt)
    # chunk-major x^T: xtc[u, p, dc*128+jj] = x[u*128+jj, dc*128+p]
    xtc = np.ascontiguousarray(
        x_bf.reshape(NCH, 128, 4, 128).transpose(0, 3, 2, 1)
    ).reshape(NCH, 128, D)
    del x_bf
    lab_ch = np.asarray(labels).astype(np.uint8).reshape(NCH, 128).T  # [p, u]
    in_maps = []
    for c in range(NCORES):
        labio = np.empty((128, LBC), dtype=np.uint8)
        labio[:, 0:NCH] = lab_ch
        labio[:, BCH * c : BCH * (c + 1)] = int(SENT)
        labio[:, NCH : NCH + BCH] = lab_ch[:, BCH * c : BCH * (c + 1)]
        xt8 = xtc[BCH * c : BCH * (c + 1)]
        if fp8:
            xb = xt8.view(np.uint8).ravel()
            flat = np.concatenate([xb, labio.ravel()]).view(xdt)[None, :]
            in_maps.append({"xlab": flat})
        else:
            in_maps.append({"xt8": xt8, "labio": labio})
    return in_maps


def kernel(features: np.ndarray, labels: np.ndarray) -> np.ndarray:
    from concourse.bass_utils import run_bass_kernel_spmd

    _enable_jax_executable_cache()
    if "nc" not in _CACHE:
        _CACHE["nc"] = _build_bass()
    nc = _CACHE["nc"]
    in_maps = _prep_inputs(features, labels)
    res = run_bass_kernel_spmd(nc, in_maps, list(range(NCORES)))
    total = sum(float(r["loss"][0, 0]) for r in res.results)
    return np.float32(total / B)
